# revision 9
# baseline (speedup 1.0000x reference)
"""Trainium2 Bass kernel for nn_M2DBlock (2D selective-scan block).

Sharding: d_inner (E=256) split 8 ways -> 32 channels/core; every core
processes both batches and all H,W. Each core computes a partial of the
final out-projection (sum over its 32 channels); host sums the 8 partials.

Per-core layout:
  tokens t = b*4096 + i*64 + w   (T = 8192)
  pairs  p = e_rel*16 + n within a 128-partition group; 4 groups cover
         e_loc in [8g, 8g+8) x n in [0,16)
  packed feature tensors [F,T] stored as [128, T*F/128]:
         partition fold*F + f, fold = t // Q, col = t % Q  (Q = T*F/128)
  scan blocks: RB=16 rows, block cols = b*1024 + r*64 + w (2048 per block)

The 2D recurrence h[i,j] = dAT*h[i-1,j] + dAL*h[i,j-1] + BX uses the native
DVE tensor_tensor_scan along w per row; rows sequential.
"""

import math
import numpy as np

import concourse.bass as bass
import concourse.bacc as bacc
import concourse.tile as tile
import concourse.mybir as mybir

B, H, W, DM = 2, 64, 64, 128
E, N, RK = 256, 16, 8
T = B * H * W            # 8192
NCORES = 8
ES = E // NCORES         # 32 channels per core
NG = ES * N // 128       # 4 partition groups
CH = 512                 # phase-1 token chunk
NCHUNK = T // CH         # 16
RB = 16                  # rows per scan block
NBLK = H // RB           # 4
BC = 2 * RB * 64         # block cols = 2048
F32 = mybir.dt.float32
AF = mybir.ActivationFunctionType
ALU = mybir.AluOpType


def bcast_ap(src, reps):
    """Partition-broadcast AP: replicate a single-partition row slice."""
    return bass.AP(tensor=src.tensor, offset=src.offset,
                   ap=[[0, reps]] + [list(d) for d in src.ap[1:]])


def build_nc():
    nc = bacc.Bacc("TRN2", target_bir_lowering=False, debug=False,
                   num_devices=NCORES)

    # ---- DRAM I/O ----
    def din(name, shape):
        return nc.dram_tensor(name, shape, F32, kind="ExternalInput").ap()

    xT = din("xT", [DM, T])
    inwT = din("inwT", [DM, E])
    inb = din("inb", [DM, 2])
    xpw0 = din("xpw0", [128, 48])
    xpw1 = din("xpw1", [128, 48])
    xpbT = din("xpbT", [8, 1])
    xpbL = din("xpbL", [8, 1])
    xpbBC = din("xpbBC", [32, 1])
    dtTwT = din("dtTwT", [RK, ES])
    dtLwT = din("dtLwT", [RK, ES])
    dtTb = din("dtTb", [ES, 1])
    dtLb = din("dtLb", [ES, 1])
    scaleAT = din("scaleAT", [128, NG])
    scaleAL = din("scaleAL", [128, NG])
    msel4 = din("msel4", [128, 4 * ES])
    dloc = din("dloc", [ES, 1])
    outwT = din("outwT", [ES, DM])
    outT = nc.dram_tensor("outT", [DM, T], F32, kind="ExternalOutput").ap()

    # DRAM scratch
    u_loc_dram = nc.dram_tensor("u_loc_scratch", [ES, T], F32).ap()

    from contextlib import ExitStack
    with tile.TileContext(nc) as tc, ExitStack() as stk:
        consts = stk.enter_context(tc.tile_pool(name="consts", bufs=1))
        resident = stk.enter_context(tc.tile_pool(name="resident", bufs=1))

        # ---- load constants ----
        def cload(ap_dram, shape, tag):
            t = consts.tile(shape, F32, tag=tag, name=tag)
            nc.sync.dma_start(out=t, in_=ap_dram)
            return t

        inwT_sb = cload(inwT, [DM, E], "inwT")
        inb_sb = cload(inb, [DM, 2], "inb")
        xpw0_sb = cload(xpw0, [128, 48], "xpw0")
        xpw1_sb = cload(xpw1, [128, 48], "xpw1")
        xpbT_sb = cload(xpbT, [8, 1], "xpbT")
        xpbL_sb = cload(xpbL, [8, 1], "xpbL")
        xpbBC_sb = cload(xpbBC, [32, 1], "xpbBC")
        dtTwT_sb = cload(dtTwT, [RK, ES], "dtTwT")
        dtLwT_sb = cload(dtLwT, [RK, ES], "dtLwT")
        dtTb_sb = cload(dtTb, [ES, 1], "dtTb")
        dtLb_sb = cload(dtLb, [ES, 1], "dtLb")
        scaleAT_sb = cload(scaleAT, [128, NG], "scaleAT")
        scaleAL_sb = cload(scaleAL, [128, NG], "scaleAL")
        msel4_sb = cload(msel4, [128, 4 * ES], "msel4")
        dloc_sb = cload(dloc, [ES, 1], "dloc")
        outwT_sb = cload(outwT, [ES, DM], "outwT")
        zeros128 = consts.tile([128, 128], F32, tag="zeros")
        nc.vector.memset(zeros128, 0.0)

        # resident packed tensors
        dTp = resident.tile([128, 2048], F32, tag="dTp")
        dLp = resident.tile([128, 2048], F32, tag="dLp")
        Gp = resident.tile([128, 2048], F32, tag="Gp")
        Bmp = resident.tile([128, 1024], F32, tag="Bmp")
        Cmp = resident.tile([128, 1024], F32, tag="Cmp")
        # per-group h tiles (reused in place across blocks)
        h_g = [resident.tile([128, BC], F32, tag=f"h{g}", name=f"h{g}")
               for g in range(NG)]
        y2p = resident.tile([128, 2048], F32, tag="y2p")

        # ---- phase 1: projections, chunked over tokens ----
        with tc.tile_pool(name="p1", bufs=3) as p1, \
             tc.tile_pool(name="p1ps", bufs=2, space="PSUM") as p1ps, \
             tc.tile_pool(name="p1psb", bufs=1, space="PSUM") as p1psb, \
             tc.tile_pool(name="p1loc", bufs=1) as p1loc:
            ulp = p1loc.tile([128, 2048], F32, tag="ulp")  # u_loc packed
            for c in range(NCHUNK):
                t0 = c * CH
                xc = p1.tile([DM, CH], F32, tag="xc")
                nc.sync.dma_start(out=xc, in_=xT[:, t0:t0 + CH])
                u = []
                for half in range(2):
                    ps = p1ps.tile([128, CH], F32, tag="psu")
                    nc.tensor.matmul(ps, inwT_sb[:, half * 128:(half + 1) * 128],
                                     xc, start=True, stop=True)
                    uh = p1.tile([128, CH], F32, tag=f"u{half}")
                    nc.scalar.activation(uh, ps, AF.Gelu,
                                         bias=inb_sb[:, half:half + 1])
                    u.append(uh)
                # dbc features in three base-0 psum tiles (K = 256, 2 halves)
                psdT = p1psb.tile([8, CH], F32, tag="psdT")
                psdL = p1psb.tile([8, CH], F32, tag="psdL")
                psbc = p1psb.tile([32, CH], F32, tag="psbc")
                for (lo, hi, pst) in ((0, 8, psdT), (8, 16, psdL),
                                      (16, 48, psbc)):
                    nc.tensor.matmul(pst, xpw0_sb[:, lo:hi], u[0],
                                     start=True, stop=False)
                    nc.tensor.matmul(pst, xpw1_sb[:, lo:hi], u[1],
                                     start=False, stop=True)
                dTs = p1.tile([8, CH], F32, tag="dTs")
                nc.scalar.activation(dTs, psdT, AF.Identity, bias=xpbT_sb)
                dLs = p1.tile([8, CH], F32, tag="dLs")
                nc.scalar.activation(dLs, psdL, AF.Identity, bias=xpbL_sb)
                BmCm = p1.tile([32, CH], F32, tag="BmCm")
                nc.scalar.activation(BmCm, psbc, AF.Identity, bias=xpbBC_sb)
                # pack Bm/Cm: fold = c//2, q0 = (c%2)*512
                f16, q16 = c // 2, (c % 2) * CH
                nc.sync.dma_start(out=Bmp[f16 * 16:(f16 + 1) * 16, q16:q16 + CH],
                                  in_=BmCm[0:16, :])
                nc.sync.dma_start(out=Cmp[f16 * 16:(f16 + 1) * 16, q16:q16 + CH],
                                  in_=BmCm[16:32, :])
                # deltas (local 32 channels)
                f32_, q32 = c // 4, (c % 4) * CH
                for wT, bT, rhs_s, dst in ((dtTwT_sb, dtTb_sb, dTs, dTp),
                                           (dtLwT_sb, dtLb_sb, dLs, dLp)):
                    psdt = p1ps.tile([ES, CH], F32, tag="psdt")
                    nc.tensor.matmul(psdt, wT, rhs_s, start=True, stop=True)
                    dsb = p1.tile([ES, CH], F32, tag="dsb")
                    nc.scalar.activation(dsb, psdt, AF.Identity, bias=bT)
                    nc.sync.dma_start(
                        out=dst[f32_ * 32:(f32_ + 1) * 32, q32:q32 + CH],
                        in_=dsb)
                # u_loc: channels 0..32 of half 0
                nc.sync.dma_start(out=u_loc_dram[:, t0:t0 + CH],
                                  in_=u[0][0:ES, :])
                nc.sync.dma_start(out=ulp[f32_ * 32:(f32_ + 1) * 32, q32:q32 + CH],
                                  in_=u[0][0:ES, :])
            # softplus(x) = ln(exp(x) + 1), in place on packed preacts
            for dp in (dTp, dLp):
                nc.scalar.activation(dp, dp, AF.Exp)
                nc.scalar.activation(dp, dp, AF.Ln, bias=1.0)
            # G = (deltaT + deltaL) * u_loc   (packed layout)
            ddt = p1loc.tile([128, 2048], F32, tag="ddt")
            nc.vector.tensor_add(ddt, dTp, dLp)
            nc.vector.tensor_mul(Gp, ddt, ulp)

        # ---- phase 2+3: scan blocks ----
        with tc.tile_pool(name="rep", bufs=2) as rep, \
             tc.tile_pool(name="gen", bufs=2) as gen, \
             tc.tile_pool(name="rt", bufs=2) as rtp, \
             tc.tile_pool(name="rowp", bufs=4) as rowp, \
             tc.tile_pool(name="p3", bufs=2) as p3, \
             tc.tile_pool(name="p2ps", bufs=1, space="PSUM") as p2ps, \
             tc.tile_pool(name="p3ps", bufs=2, space="PSUM") as p3ps:
            for blk in range(NBLK):
                i0 = blk * RB
                # shared Bm/Cm expansions for this block
                Brep = rep.tile([128, BC], F32, tag="Brep")
                Cmrep = rep.tile([128, BC], F32, tag="Cmrep")
                for dst, srcp in ((Brep, Bmp), (Cmrep, Cmp)):
                    for b in range(2):
                        fold = 4 * b + blk
                        src = srcp[fold * 16:(fold + 1) * 16, 0:1024]
                        for e_rel in range(8):
                            nc.sync.dma_start(
                                out=dst[e_rel * 16:(e_rel + 1) * 16,
                                        b * 1024:(b + 1) * 1024],
                                in_=src)
                psy = [p2ps.tile([ES, 512], F32, tag=f"psy{qc}", name=f"psy{qc}")
                       for qc in range(4)]
                for g in range(NG):
                    dAT = gen.tile([128, BC], F32, tag="dAT")
                    dAL = gen.tile([128, BC], F32, tag="dAL")
                    BX = gen.tile([128, BC], F32, tag="BX")
                    # expand deltas / G into tiles, then transform in place
                    for dst, srcp in ((dAT, dTp), (dAL, dLp), (BX, Gp)):
                        for b in range(2):
                            p_base = (2 * b + blk // 2) * 32
                            q0 = (blk % 2) * 1024
                            src = srcp[p_base + 8 * g:p_base + 8 * g + 8,
                                       q0:q0 + 1024]
                            repap = bass.AP(tensor=src.tensor,
                                            offset=src.offset,
                                            ap=[list(src.ap[0]), [0, 16],
                                                list(src.ap[1])])
                            nc.sync.dma_start(
                                out=dst[:, b * 1024:(b + 1) * 1024], in_=repap)
                    nc.scalar.activation(dAT, dAT, AF.Exp,
                                         scale=scaleAT_sb[:, g:g + 1])
                    nc.scalar.activation(dAL, dAL, AF.Exp,
                                         scale=scaleAL_sb[:, g:g + 1])
                    nc.gpsimd.tensor_mul(BX, BX, Brep)
                    # row scans
                    hg = h_g[g]
                    hv = hg.rearrange("p (b r w) -> p b r w", b=2, r=RB)
                    dATv = dAT.rearrange("p (b r w) -> p b r w", b=2, r=RB)
                    dALv = dAL.rearrange("p (b r w) -> p b r w", b=2, r=RB)
                    BXv = BX.rearrange("p (b r w) -> p b r w", b=2, r=RB)
                    for r in range(RB):
                        d1 = rowp.tile([128, 128], F32, tag="d1")
                        d1v = d1.rearrange("p (b w) -> p b w", b=2)
                        if blk == 0 and r == 0:
                            hprev = zeros128.rearrange("p (b w) -> p b w", b=2)
                        else:
                            hprev = hv[:, :, (r - 1) % RB, :]
                        nc.vector.tensor_mul(d1v, dATv[:, :, r, :], hprev)
                        nc.vector.tensor_add(d1v, d1v, BXv[:, :, r, :])
                        for b in range(2):
                            nc.vector.tensor_tensor_scan(
                                hv[:, b, r, :], dALv[:, b, r, :], d1v[:, b, :],
                                0.0, ALU.mult, ALU.add)
                    # readout: rT = h * Cm_rep, then 0/1-select matmul sums n
                    rT = rtp.tile([128, BC], F32, tag="rT")
                    nc.gpsimd.tensor_mul(rT, hg, Cmrep)
                    for qc in range(4):
                        nc.tensor.matmul(psy[qc], msel4_sb[:, g * ES:(g + 1) * ES],
                                         rT[:, qc * 512:(qc + 1) * 512],
                                         start=(g == 0), stop=(g == NG - 1))
                # y2 = u*D + psy, packed for the deferred output phase
                for qc in range(4):
                    b = qc // 2
                    off = b * 4096 + i0 * 64 + (qc % 2) * 512
                    ub = p3.tile([ES, 512], F32, tag="ub")
                    nc.sync.dma_start(out=ub, in_=u_loc_dram[:, off:off + 512])
                    y2 = p3.tile([ES, 512], F32, tag="y2")
                    nc.vector.scalar_tensor_tensor(
                        y2, ub, dloc_sb, psy[qc], ALU.mult, ALU.add)
                    fo, qo = off // 2048, off % 2048
                    nc.sync.dma_start(
                        out=y2p[fo * 32:(fo + 1) * 32, qo:qo + 512], in_=y2)
            # ---- output phase: gelu(y2) and out-projection ----
            nc.scalar.activation(y2p, y2p, AF.Gelu)
            for c in range(NCHUNK):
                t0 = c * CH
                fo, qo = t0 // 2048, t0 % 2048
                gy = p3.tile([ES, 512], F32, tag="gy")
                nc.sync.dma_start(out=gy,
                                  in_=y2p[fo * 32:(fo + 1) * 32, qo:qo + 512])
                pso = p3ps.tile([DM, 512], F32, tag="pso")
                nc.tensor.matmul(pso, outwT_sb, gy, start=True, stop=True)
                osb = p3.tile([DM, 512], F32, tag="osb")
                nc.scalar.copy(osb, pso)
                nc.sync.dma_start(out=outT[:, t0:t0 + CH], in_=osb)

    nc.compile()
    return nc


def shard_inputs(inputs):
    """Host-side prep: returns list of 8 per-core input dicts."""
    x = np.ascontiguousarray(np.asarray(inputs["x"], np.float32))
    in_proj_w = np.asarray(inputs["in_proj_w"], np.float32)
    in_proj_b = np.asarray(inputs["in_proj_b"], np.float32)
    x_proj_w = np.asarray(inputs["x_proj_w"], np.float32)
    x_proj_b = np.asarray(inputs["x_proj_b"], np.float32)
    dtT_w = np.asarray(inputs["dtT_w"], np.float32)
    dtT_b = np.asarray(inputs["dtT_b"], np.float32)
    dtL_w = np.asarray(inputs["dtL_w"], np.float32)
    dtL_b = np.asarray(inputs["dtL_b"], np.float32)
    AT_log = np.asarray(inputs["AT_log"], np.float32)
    AL_log = np.asarray(inputs["AL_log"], np.float32)
    D = np.asarray(inputs["D"], np.float32)
    out_w = np.asarray(inputs["out_w"], np.float32)

    xT = np.ascontiguousarray(x.reshape(T, DM).T)
    msel4 = np.zeros((128, 4 * ES), np.float32)
    for g in range(NG):
        for p in range(128):
            msel4[p, ES * g + 8 * g + p // 16] = 1.0

    in_maps = []
    for c in range(NCORES):
        e0 = ES * c
        perm = (np.arange(E) + e0) % E
        sAT = np.zeros((128, NG), np.float32)
        sAL = np.zeros((128, NG), np.float32)
        for g in range(NG):
            for p in range(128):
                e_loc, n = 8 * g + p // 16, p % 16
                sAT[p, g] = -math.exp(AT_log[e0 + e_loc, n])
                sAL[p, g] = -math.exp(AL_log[e0 + e_loc, n])
        xpwT = np.ascontiguousarray(x_proj_w[:, perm].T)  # [256, 48]
        in_maps.append({
            "xT": xT,
            "inwT": np.ascontiguousarray(in_proj_w[perm].T),
            "inb": np.ascontiguousarray(in_proj_b[perm].reshape(2, 128).T),
            "xpw0": np.ascontiguousarray(xpwT[0:128]),
            "xpw1": np.ascontiguousarray(xpwT[128:256]),
            "xpbT": np.ascontiguousarray(x_proj_b[0:8].reshape(8, 1)),
            "xpbL": np.ascontiguousarray(x_proj_b[8:16].reshape(8, 1)),
            "xpbBC": np.ascontiguousarray(x_proj_b[16:48].reshape(32, 1)),
            "dtTwT": np.ascontiguousarray(dtT_w[e0:e0 + ES].T),
            "dtLwT": np.ascontiguousarray(dtL_w[e0:e0 + ES].T),
            "dtTb": np.ascontiguousarray(dtT_b[e0:e0 + ES].reshape(ES, 1)),
            "dtLb": np.ascontiguousarray(dtL_b[e0:e0 + ES].reshape(ES, 1)),
            "scaleAT": sAT,
            "scaleAL": sAL,
            "msel4": msel4,
            "dloc": np.ascontiguousarray(D[e0:e0 + ES].reshape(ES, 1)),
            "outwT": np.ascontiguousarray(out_w[:, e0:e0 + ES].T),
        })
    return in_maps


_NC_CACHE = None


def _get_nc():
    global _NC_CACHE
    if _NC_CACHE is None:
        _NC_CACHE = build_nc()
    return _NC_CACHE


def kernel(**inputs):
    from concourse.bass_utils import run_bass_kernel_spmd
    nc = _get_nc()
    in_maps = shard_inputs(inputs)
    res = run_bass_kernel_spmd(nc, in_maps, core_ids=list(range(NCORES)))
    acc = np.zeros((DM, T), np.float64)
    for r in res.results:
        acc += r["outT"].astype(np.float64)
    out = acc.T.astype(np.float32) + np.asarray(inputs["out_b"], np.float32)
    return out.reshape(B, H, W, DM).astype(np.float32)
